# revision 1
# baseline (speedup 1.0000x reference)
"""AttentionPooling Trainium2 kernel (v2, optimized).

Problem (per full input):
    hidden [B=8, S=8192, DM=1024] f32, mask [B, S] bool, query [K=8, DM] f32
    logits = einsum('kd,bsd->bks', query, hidden); masked (-1e4) softmax over S
    out    = einsum('bks,bsd->bkd', attn, hidden)              -> [B, K, DM] f32

Sharding: data-parallel over batch B; core i handles batch i. No collectives.

Design (212us baseline -> ~35us, DMA-bound):
  1. Mask compaction on host: masked positions have softmax weight exactly 0,
     so only the ~50% unmasked rows are shipped (gathered on host, padded to
     a multiple of 256 with zero rows whose weight underflows to 0).
  2. fp16 single copy of ONLY the transposed layout hT [DM, S_pad] (~9
     MB/core vs 64 MB in v1).  Validated numerically: fp16 logits + bf16
     attn weights + bf16 weighted-sum operand give rel err ~8e-3 vs the
     2e-2 gate.
  3. Both matmuls use h blocks as the STATIONARY operand with tiny moving
     operands, so each 128x128 block costs only 8 PE cycles:
       mm1: L^T[s,k] = blk(hT)[d,s]^T @ qT[d,k]   (accumulate over 8 d-chunks)
       mm2: oT[d,k] += hnat[s,d]^T @ p[s,k]       (accumulate over all chunks)
     The natural-layout blocks hnat are produced on-chip by PE transposes
     into PSUM and copied to SBUF by the DVE (fp16 -> bf16 in the copy).
  4. The per-row softmax shift M (host-estimated bound from 512 sampled
     logits + 30 margin) is folded into the logit accumulation as a
     ones-row x (-M) matmul.  p stays in bf16: fp32 exponent range means
     sampling error in M can never over/underflow p.
  5. PSUM discipline: start_tensor_calc zeroes a whole zero region, so each
     PSUM accumulator bank gets exactly ONE start and ONE stop (a start per
     j-region would wipe neighbours' partial sums).
  6. Software pipelining (LAG=2): the in-order PE consumes (hnat, p) from
     two chunks back so mm2 never stalls behind the transpose->copy chain.
  7. Ends: all constants ship as one packed DMA on the Act queue (overlaps
     the first hT tile on the SP queue); the Exp activation table is warmed
     during the prologue; the finalize divides via a DVE 32x32
     stream-transpose of 1/denom + a ones-row broadcast matmul + a single
     broadcast tensor_mul straight out of PSUM.  Output ships as out^T
     [128, 64]; the host only re-lays-out (no arithmetic off-device).
"""

import sys

import numpy as np

sys.path.insert(0, "/opt/trn_rl_repo")

import ml_dtypes

import concourse.tile as tile
from concourse import bacc, mybir

FP = mybir.dt.float32
F16 = mybir.dt.float16
BF = mybir.dt.bfloat16
F16_NP = np.float16
BF_NP = ml_dtypes.bfloat16

# Problem config (hardcoded; harness calls kernel() with exactly these shapes)
B, S, DM, K = 8, 8192, 1024, 8
N_CORES = 8
NCD = DM // 128      # 8 d-chunks
ST = 256             # s elements per DMA tile (innermost contiguous run, 512B)


def build_program(n_st):
    """Per-core Bass program for n_st s-tiles of 256 (n_ch = 2*n_st chunks
    of 128).  Returns the compiled Bacc module."""
    n_ch = 2 * n_st

    nc = bacc.Bacc(
        "TRN2",
        target_bir_lowering=False,
        debug=False,
        num_devices=N_CORES,
    )

    hT_pack = nc.dram_tensor(
        "hT_pack", [n_st, NCD, 128, ST], F16, kind="ExternalInput"
    ).ap()
    cpack = nc.dram_tensor(
        "cpack", [128, NCD * K + 128 + K], F16, kind="ExternalInput"
    ).ap()
    out = nc.dram_tensor("out", [128, NCD * K], FP, kind="ExternalOutput").ap()

    with tile.TileContext(nc) as tc:
        with (
            tc.tile_pool(name="const", bufs=1) as const_pool,
            tc.tile_pool(name="state", bufs=1) as state_pool,
            tc.tile_pool(name="hT", bufs=4) as hT_pool,
            tc.tile_pool(name="hnat", bufs=4) as hnat_pool,
            tc.tile_pool(name="psL", bufs=2, space="PSUM") as psL_pool,
            tc.tile_pool(name="psT", bufs=3, space="PSUM") as psT_pool,
            tc.tile_pool(name="psO", bufs=1, space="PSUM") as psO_pool,
            tc.tile_pool(name="ptile", bufs=4) as p_pool,
            tc.tile_pool(name="small", bufs=4) as small_pool,
        ):
            # ---- constants: ONE packed DMA on the Activation HWDGE queue,
            # overlapping the SP queue's first hT tile ----
            cp_sb = const_pool.tile([128, NCD * K + 128 + K], F16, tag="cpack")
            nc.scalar.dma_start(out=cp_sb[:], in_=cpack)
            qT_sb = cp_sb[:, : NCD * K]
            id16_sb = cp_sb[:, NCD * K : NCD * K + 128]
            negM_sb = cp_sb[0:1, NCD * K + 128 : NCD * K + 128 + K]
            ones_row = const_pool.tile([1, 128], F16, tag="ones_row")
            nc.vector.memset(ones_row[:], 1.0)
            ones_row_f = const_pool.tile([1, 128], FP, tag="ones_row_f")
            nc.vector.memset(ones_row_f[:], 1.0)
            ones_col = const_pool.tile([128, 1], BF, tag="ones_col")
            nc.vector.memset(ones_col[:], 1.0)

            # Warm the Exp activation table during the DMA prologue so the
            # first chunk's exp doesn't eat the 1.3us table load.
            warm_in = const_pool.tile([1, 1], FP, tag="warm_in")
            nc.vector.memset(warm_in[:], 0.0)
            warm_out = const_pool.tile([1, 1], FP, tag="warm_out")
            nc.scalar.activation(
                warm_out[:], warm_in[:], mybir.ActivationFunctionType.Exp
            )

            # ---- persistent accumulators ----
            oT = psO_pool.tile([128, NCD * K], FP, tag="oT")   # [d%128, j*8+k]
            dn = psO_pool.tile([K, 1], FP, tag="dn")

            # GPSIMD cannot access PSUM; all PSUM->SBUF copies go to the
            # DVE (2-byte 2x mode keeps 34 copies ~22us < DMA 25us), leaving
            # the Activation engine free for the exp chain.
            copy_engines = [lambda o, i: nc.vector.tensor_copy(o, i)]

            # Software pipelining: the PE consumes (hnat, p) from LAG chunks
            # ago, so mm2 never stalls the in-order PE behind the
            # transpose -> PSUM->SBUF copy chain of the same chunk.
            LAG = 2
            pending = []

            def emit_mm2(hnat, p_t, cs):
                # NOTE: start_tensor_calc zeroes the whole PSUM zero region,
                # so only the FIRST matmul into the oT bank may set start (a
                # start per j would wipe the earlier js' chunk-0
                # contributions).  One start, one stop per bank.
                for j in range(NCD):
                    nc.tensor.matmul(
                        oT[:, j * K : (j + 1) * K],
                        hnat[:, j * 128 : (j + 1) * 128],
                        p_t[:],
                        start=(cs == 0 and j == 0),
                        stop=(cs == n_ch - 1 and j == NCD - 1),
                    )
                nc.tensor.matmul(
                    dn[:],
                    p_t[:],
                    ones_col[:],
                    start=(cs == 0),
                    stop=(cs == n_ch - 1),
                )

            for t in range(n_st):
                hT_t = hT_pool.tile([128, NCD * ST], F16, tag="hT")
                if t == 0 or t == n_st - 1:
                    h = NCD // 2
                    for d in range(2):
                        nc.sync.dma_start(
                            out=hT_t[:, d * h * ST : (d + 1) * h * ST].rearrange(
                                "p (j s) -> p j s", j=h
                            ),
                            in_=hT_pack[t, d * h : (d + 1) * h].rearrange(
                                "j p s -> p j s"
                            ),
                        )
                else:
                    nc.sync.dma_start(
                        out=hT_t[:].rearrange("p (j s) -> p j s", j=NCD),
                        in_=hT_pack[t].rearrange("j p s -> p j s"),
                    )
                for c in range(2):
                    cs = 2 * t + c

                    def blk(j):
                        base = j * ST + c * 128
                        return hT_t[:, base : base + 128]

                    # ---- mm1: L^T[s,k] = sum_d h[s,d] q[k,d] - M_k ----
                    Lt = psL_pool.tile([128, K], FP, tag="Lt")
                    for j in range(NCD):
                        nc.tensor.matmul(
                            Lt[:],
                            blk(j),
                            qT_sb[:, j * K : (j + 1) * K],
                            start=(j == 0),
                            stop=False,
                        )
                    nc.tensor.matmul(
                        Lt[:],
                        ones_row[:],
                        negM_sb,
                        start=False,
                        stop=True,
                    )

                    # ---- transpose hT blocks -> natural layout (PSUM) ----
                    psT = psT_pool.tile([128, NCD * 128], F16, tag="psT")
                    for j in range(NCD):
                        nc.tensor.transpose(
                            psT[:, j * 128 : (j + 1) * 128], blk(j), id16_sb
                        )

                    # ---- p = exp(L^T - M), bf16 ----
                    p_t = p_pool.tile([128, K], BF, tag="p")
                    nc.scalar.activation(
                        p_t[:], Lt[:], mybir.ActivationFunctionType.Exp
                    )

                    # ---- copy natural blocks PSUM -> SBUF (bf16) ----
                    hnat = hnat_pool.tile([128, NCD * 128], BF, tag="hnat")
                    copy_engines[0](hnat[:], psT[:])

                    # ---- mm2 for the chunk LAG back ----
                    pending.append((hnat, p_t, cs))
                    if len(pending) > LAG:
                        emit_mm2(*pending.pop(0))

            for args in pending:
                emit_mm2(*args)

            # ---- finalize: out_sb[p, j*K+k] = oT[p, j*K+k] / dn[k] ----
            # (outT orientation is shipped as-is; the host wrapper only
            # re-lays-out [128, NCD*K] -> [K, DM], no arithmetic)
            # reciprocal into a zero-padded [32, 32] block, then a DVE
            # 32x32 stream-transpose puts 1/dn on row 0; a ones-row matmul
            # broadcasts it across 128 partitions (PSUM, DVE-readable).
            rsq = small_pool.tile([32, 32], FP, tag="rsq")
            nc.vector.memset(rsq[:], 1.0)
            nc.vector.reciprocal(rsq[0:K, 0:1], dn[:])
            rsqT = small_pool.tile([32, 32], FP, tag="rsqT")
            nc.vector.transpose(rsqT[:], rsq[:])
            rb_ps = psT_pool.tile([128, K], FP, name="rb_ps", tag="psT")
            nc.tensor.matmul(
                rb_ps[:], ones_row_f[:], rsqT[0:1, 0:K], start=True, stop=True
            )
            # TensorTensor may read only one input from PSUM: rb -> SBUF
            rb_sb = small_pool.tile([128, K], FP, tag="rb_sb")
            nc.vector.tensor_copy(rb_sb[:], rb_ps[:])
            out_sb = state_pool.tile([128, NCD * K], FP, tag="out_sb")
            nc.vector.tensor_mul(
                out_sb[:].rearrange("p (r k) -> p r k", r=NCD),
                oT[:].rearrange("p (r k) -> p r k", r=NCD),
                rb_sb[:, None, :].broadcast_to([128, NCD, K]),
            )
            nc.sync.dma_start(out=out, in_=out_sb[:])

    nc.compile()
    return nc


_CACHED = {}


def _get_program(n_st):
    if n_st not in _CACHED:
        _CACHED[n_st] = build_program(n_st)
    return _CACHED[n_st]


def make_in_maps(hidden, mask, query):
    """Host staging: compact unmasked rows, fp16 convert, pack layouts."""
    hidden = np.ascontiguousarray(hidden, dtype=np.float32)
    mask = np.asarray(mask)
    query = np.asarray(query, dtype=np.float32)
    b, s, dm = hidden.shape
    k = query.shape[0]

    q16 = query.astype(F16_NP)                       # [K, DM]
    qT_pack = (
        q16.T.reshape(NCD, 128, k).transpose(1, 0, 2).reshape(128, NCD * k)
    )
    ident16 = np.eye(128, dtype=F16_NP)

    idxs = [np.flatnonzero(mask[i]) for i in range(b)]
    n_max = max(1, max(len(ix) for ix in idxs))
    s_pad = ((n_max + ST - 1) // ST) * ST
    n_st = s_pad // ST

    rngM = np.random.default_rng(12345)
    in_maps = []
    for i in range(b):
        ix = idxs[i]
        n_i = len(ix)
        hc = np.zeros((s_pad, dm), dtype=F16_NP)
        hc[:n_i] = hidden[i][ix]
        # Per-row exp-shift bound M from sampled logits (+30 margin).  bf16 p
        # tolerates a loose bound in both directions.
        nsamp = min(512, max(n_i, 1))
        if n_i > 0:
            smp = rngM.choice(n_i, nsamp, replace=False)
            ls = query @ hidden[i][ix[smp]].T        # [K, nsamp]
            M = np.maximum(ls.max(axis=1) + 30.0, 60.0)
        else:
            M = np.full(k, 60.0)
        negM = (-M).astype(F16_NP)
        cpack = np.zeros((128, NCD * k + 128 + k), dtype=F16_NP)
        cpack[:, : NCD * k] = qT_pack
        cpack[:, NCD * k : NCD * k + 128] = ident16
        cpack[0, NCD * k + 128 :] = negM

        hT = np.ascontiguousarray(hc.T)              # [DM, s_pad]
        hT_pack = np.ascontiguousarray(
            hT.reshape(NCD, 128, n_st, ST).transpose(2, 0, 1, 3)
        )
        in_maps.append({"hT_pack": hT_pack, "cpack": cpack})
    return n_st, in_maps


class _Runner:
    """jit-once SPMD runner (mirrors bass2jax.run_bass_via_pjrt, but reusable
    across calls so repeated invocations don't re-trace/re-compile)."""

    def __init__(self, nc):
        import jax
        from jax.sharding import Mesh, PartitionSpec, NamedSharding
        from jax.experimental.shard_map import shard_map
        from concourse.bass2jax import (
            _bass_exec_p,
            install_neuronx_cc_hook,
            partition_id_tensor,
        )

        install_neuronx_cc_hook()
        self.jax = jax
        partition_name = (
            nc.partition_id_tensor.name if nc.partition_id_tensor else None
        )
        in_names, out_names, out_avals, zero_outs = [], [], [], []
        for alloc in nc.m.functions[0].allocations:
            if not isinstance(alloc, mybir.MemoryLocationSet):
                continue
            name = alloc.memorylocations[0].name
            if alloc.kind == "ExternalInput":
                if name != partition_name:
                    in_names.append(name)
            elif alloc.kind == "ExternalOutput":
                out_names.append(name)
                shape = tuple(alloc.tensor_shape)
                dtype = mybir.dt.np(alloc.dtype)
                out_avals.append(jax.core.ShapedArray(shape, dtype))
                zero_outs.append(np.zeros(shape, dtype))
        self.in_names, self.out_names = in_names, out_names
        self.out_avals, self.zero_outs = out_avals, zero_outs
        n_params, n_outs = len(in_names), len(out_names)
        all_in_names = in_names + out_names
        if partition_name is not None:
            all_in_names = all_in_names + [partition_name]
        all_in_names = tuple(all_in_names)

        def _body(*args):
            operands = list(args)
            if partition_name is not None:
                operands.append(partition_id_tensor())
            outs = _bass_exec_p.bind(
                *operands,
                out_avals=tuple(out_avals),
                in_names=all_in_names,
                out_names=tuple(out_names),
                lowering_input_output_aliases=(),
                sim_require_finite=True,
                sim_require_nnan=True,
                nc=nc,
            )
            return tuple(outs)

        devices = jax.devices()[:N_CORES]
        self.mesh = Mesh(np.asarray(devices), ("core",))
        in_specs = (PartitionSpec("core"),) * (n_params + n_outs)
        out_specs = (PartitionSpec("core"),) * n_outs
        self.fn = jax.jit(
            shard_map(
                _body,
                mesh=self.mesh,
                in_specs=in_specs,
                out_specs=out_specs,
                check_rep=False,
            ),
            donate_argnums=tuple(range(n_params, n_params + n_outs)),
            keep_unused=True,
        )
        self.sharding = NamedSharding(self.mesh, PartitionSpec("core"))
        self._dev_in = None
        self._dev_in_key = None

    def put_inputs(self, in_maps):
        key = id(in_maps)
        if self._dev_in_key == key:
            return self._dev_in
        concat_in = [
            np.concatenate([m[name] for m in in_maps], axis=0)
            for name in self.in_names
        ]
        self._dev_in = [self.jax.device_put(x, self.sharding) for x in concat_in]
        self._dev_in_key = key
        return self._dev_in

    def run(self, in_maps):
        dev_in = self.put_inputs(in_maps)
        dev_zero = [
            self.jax.device_put(
                np.zeros((N_CORES * z.shape[0], *z.shape[1:]), z.dtype),
                self.sharding,
            )
            for z in self.zero_outs
        ]
        outs = self.fn(*dev_in, *dev_zero)
        self.jax.block_until_ready(outs)
        return {
            name: np.asarray(outs[i]).reshape(
                N_CORES, *self.out_avals[i].shape
            )
            for i, name in enumerate(self.out_names)
        }


_RUNNERS = {}


def _get_runner(n_st):
    if n_st not in _RUNNERS:
        _RUNNERS[n_st] = _Runner(_get_program(n_st))
    return _RUNNERS[n_st]


def kernel(hidden, mask, query):
    n_st, in_maps = make_in_maps(hidden, mask, query)
    runner = _get_runner(n_st)
    outT = runner.run(in_maps)["out"]          # [B, 128, NCD*K], = out^T/dn
    out = outT.reshape(B, 128, NCD, K).transpose(0, 3, 2, 1).reshape(B, K, DM)
    return np.ascontiguousarray(out, dtype=np.float32)

